# revision 13
# baseline (speedup 1.0000x reference)
"""TRN2 Bass kernel for nn_ClassSemantic (scatter_memory).

Strategy
--------
Data-parallel over batch: core k owns samples 4k..4k+3 and runs
projection (1x1 conv) + memory-gather attention + concat.

fp16 end-to-end on the device: the baseline fp32r kernel was limited by
the PE streaming 4-byte rhs rows from SBUF (matmuls measured 383 ns vs
213 ns ideal for 512 rows) and by power throttling (50%-util windows,
87.7 us) driven by ~2.67 TB/s chip HBM traffic.  fp16 halves the SBUF
bytes per PE row, the HBM bytes (host converts in/out), and the power.
Accuracy: fp16 rounding contributes ~6e-4 scale-relative error, well
inside the 2e-2 gate (and ~2e-3 local gate).

The sequential EMA queue update depends only on per-sample masked
feature means, which are algebraically separable:
    feat_b = mean_hw((Wp@f + bp) * pred) = Wp @ mean_hw(f * pred) + bp * mean(pred)
The tiny 32-step scan runs on the host in float64 and the final queue
rows ship to every core as constants; no collective needed.

Softmax over the 20 memory slots: logits are in [-3.5, 3.5] so exp()
without max subtraction is safe.  Column sums / broadcasts across the
20-partition axis use a tiny all-ones matmul on the PE; the normalize
multiply runs on the Pool engine (SBUF-only operands - Pool cannot
read PSUM) to keep the DVE free for PSUM evictions.
"""
import os
import numpy as np
from contextlib import ExitStack

B, IN_C, H, W_SP = 32, 512, 64, 64
CODE, CLASSES, MEM = 256, 4, 20
HW = H * W_SP              # 4096
NCORES = 8
BPC = B // NCORES          # 4 samples per core
DECAY, EPS = 0.9, 1e-12
NCH = 8                    # chunks per sample
NT = HW // NCH             # 512 spatial positions per chunk
LDC = 2                    # chunks per input DMA load

_PROGRAM_CACHE = {}
LAST_RESULTS = None        # stash for test harness introspection


def _host_queue_update(feats, preds, labels, flag, queue, Wp, bp):
    """Final queue after the reference's sequential EMA scan (float64)."""
    if int(flag) != 1:
        return queue.astype(np.float32)
    f3 = feats.reshape(B, IN_C, HW)
    p2 = preds.reshape(B, HW)
    # g_b = mean_n feats_b[:, n] * pred_b[n]  (batched sgemv)
    g = np.matmul(f3, p2[:, :, None])[:, :, 0] / np.float32(HW)
    feat = g @ Wp.T.astype(np.float32) + bp[None, :] * p2.mean(axis=1)[:, None]
    q = queue.astype(np.float64)
    for i in range(B):
        l = int(labels[i])
        f = feat[i].astype(np.float64)
        slot = q[l]
        logit = slot @ f
        upd = logit[:, None] * f[None, :]
        nrm = np.sqrt((upd * upd).sum(axis=1, keepdims=True))
        upd = upd / np.maximum(nrm, EPS)
        q[l] = DECAY * slot + (1.0 - DECAY) * upd
    return q.astype(np.float32)


def _build_program():
    from concourse import bacc, mybir
    import concourse.tile as tile

    f32, f16 = mybir.dt.float32, mybir.dt.float16
    nc = bacc.Bacc("TRN2", target_bir_lowering=False, debug=False)

    feats_in = nc.dram_tensor("feats", [BPC, IN_C, HW], f16, kind="ExternalInput").ap()
    wpt_in = nc.dram_tensor("wpt", [IN_C, CODE], f16, kind="ExternalInput").ap()
    bp_in = nc.dram_tensor("bpc", [128, 2], f32, kind="ExternalInput").ap()
    qat_in = nc.dram_tensor("qat", [BPC, CODE, MEM], f16, kind="ExternalInput").ap()
    qa_in = nc.dram_tensor("qa", [BPC, MEM, CODE], f16, kind="ExternalInput").ap()
    ones_in = nc.dram_tensor("ones", [MEM, MEM], f16, kind="ExternalInput").ap()
    out_ext = nc.dram_tensor("out", [BPC, 2 * CODE, HW], f16, kind="ExternalOutput").ap()
    # channels = g*256 + h*128 + p: g=0 -> new_feat (u), g=1 -> x
    out_v = out_ext.rearrange("b (g h p) n -> b g p h n", g=2, h=2, p=128)

    with tile.TileContext(nc) as tc, ExitStack() as ctx:
        consts = ctx.enter_context(tc.tile_pool(name="consts", bufs=1))
        fpool = ctx.enter_context(tc.tile_pool(name="fpool", bufs=3))
        xpool = ctx.enter_context(tc.tile_pool(name="xpool", bufs=2))
        upool = ctx.enter_context(tc.tile_pool(name="upool", bufs=2))
        spool = ctx.enter_context(tc.tile_pool(name="spool", bufs=3))
        spool4 = ctx.enter_context(tc.tile_pool(name="spool4", bufs=5))
        # 8 PSUM banks: proj accumulators get 4 (two chunks of slack, the
        # measured stall in v2 was proj matmuls waiting on bias evictions),
        # logit/colsum 1 each, attention-output 2.
        ppp = ctx.enter_context(tc.tile_pool(name="ppp", bufs=4, space="PSUM"))
        pps = ctx.enter_context(tc.tile_pool(name="pps", bufs=1, space="PSUM"))
        ppc = ctx.enter_context(tc.tile_pool(name="ppc", bufs=1, space="PSUM"))
        ppu = ctx.enter_context(tc.tile_pool(name="ppu", bufs=2, space="PSUM"))

        # constants load on the scalar HWDGE ring so the sync ring starts
        # streaming feats immediately
        wpt_sb = consts.tile([128, 4, CODE], f16, name="wpt_sb")       # [p, kchunk, o]
        nc.scalar.dma_start(wpt_sb[:], wpt_in.rearrange("(kk p) m -> p kk m", p=128))
        bp_sb = consts.tile([128, 2], f32, name="bp_sb")               # [p, half]
        nc.scalar.dma_start(bp_sb[:], bp_in[:])
        qat_sb = consts.tile([128, BPC, 2, MEM], f16, name="qat_sb")   # [p, b, kchunk, m]
        qa_sb = consts.tile([MEM, BPC, CODE], f16, name="qa_sb")       # [m, b, c]
        ones_sb = consts.tile([MEM, MEM], f16, name="ones_sb")

        def load_attn_consts():
            nc.scalar.dma_start(qat_sb[:], qat_in.rearrange("b (kk p) m -> p b kk m", p=128))
            nc.scalar.dma_start(qa_sb[:], qa_in.rearrange("b m c -> m b c"))
            nc.scalar.dma_start(ones_sb[:], ones_in[:])

        ft_tiles = {}
        x_tiles = {}
        u_tiles = {}
        pexp_t = {}
        cs_t = {}
        rc_t = {}
        pn_t = {}
        T = BPC * NCH

        def bj(c):
            return c // NCH, c % NCH

        def proj_chunk(c):
            b, j = bj(c)
            if j == 0:
                x_tiles[b] = xpool.tile([128, 2, HW], f16, tag="x_sb", name=f"x_sb{b}")
                u_tiles[b] = upool.tile([128, 2, HW], f16, tag="u_sb", name=f"u_sb{b}")
            if c % LDC == 0:
                # one DMA covers LDC chunks: larger transfers, fewer issues.
                # The first group loads per-chunk so the PE starts ~halved
                # pipeline-fill earlier (first matmul waited 14.8us in v3).
                g = c // LDC
                feats_b = feats_in[b].rearrange("(kk p) n -> p kk n", p=128)
                ft = fpool.tile([128, 4, LDC * NT], f16, tag="ft", name=f"ft{g}")
                lo = (j // LDC) * LDC * NT
                if g == 0:
                    for q in range(LDC):
                        nc.sync.dma_start(
                            ft[:, :, q * NT:(q + 1) * NT],
                            feats_b[:, :, lo + q * NT:lo + (q + 1) * NT])
                else:
                    nc.sync.dma_start(ft[:], feats_b[:, :, lo:lo + LDC * NT])
                ft_tiles[g] = ft
            ft = ft_tiles[c // LDC]
            fs = slice((j % LDC) * NT, (j % LDC + 1) * NT)
            x_sb = x_tiles[b]
            for h in range(2):
                ps = ppp.tile([128, NT], f32, tag="proj_ps", name=f"pps{c}_{h}")
                for kk in range(4):
                    nc.tensor.matmul(
                        ps[:], wpt_sb[:, kk, h * 128:(h + 1) * 128], ft[:, kk, fs],
                        start=(kk == 0), stop=(kk == 3))
                # psum -> sbuf with per-channel bias, rounding to fp16
                if h == 0:
                    nc.scalar.activation(
                        x_sb[:, h, j * NT:(j + 1) * NT], ps[:],
                        mybir.ActivationFunctionType.Identity,
                        bias=bp_sb[:, h:h + 1])
                else:
                    nc.vector.tensor_scalar_add(
                        x_sb[:, h, j * NT:(j + 1) * NT], ps[:], bp_sb[:, h:h + 1])

        def logit_stage(c):
            b, j = bj(c)
            x_sb = x_tiles[b]
            js = slice(j * NT, (j + 1) * NT)
            lg = pps.tile([MEM, NT], f32, tag="logit_ps", name=f"lg{c}")
            for kk in range(2):
                nc.tensor.matmul(lg[:], qat_sb[:, b, kk, :], x_sb[:, kk, js],
                                 start=(kk == 0), stop=(kk == 1))
            pexp = spool4.tile([MEM, NT], f16, tag="pexp", name=f"pexp{c}")
            nc.scalar.activation(pexp[:], lg[:], mybir.ActivationFunctionType.Exp)
            pexp_t[c] = pexp

        def sum_stage(c):
            # lhsT = all-ones [20,20]: every output partition gets the
            # column sum, so no cross-partition broadcast is needed later.
            cs = ppc.tile([MEM, NT], f32, tag="colsum_ps", name=f"cs{c}")
            nc.tensor.matmul(cs[:], ones_sb[:], pexp_t[c][:], start=True, stop=True)
            cs_t[c] = cs

        def recip_stage(c):
            # 1/colsum at ~18 correct bits (more than fp16's mantissa)
            rc = spool.tile([MEM, NT], f32, tag="recip", name=f"rc{c}")
            nc.vector.reciprocal_approx_fast(out=rc[:], in_=cs_t.pop(c)[:])
            rc_t[c] = rc

        def pn_stage(c):
            # normalize on the Pool engine (SBUF-only operands), one
            # pipeline stage ahead of the u matmuls so the PE never waits
            # on the Pool latency (v2 measured 444 ns of wait here)
            rc = rc_t.pop(c)
            pn = spool.tile([MEM, NT], f16, tag="pn", name=f"pn{c}")
            nc.gpsimd.tensor_mul(pn[:], pexp_t.pop(c)[:], rc[:])
            pn_t[c] = pn

        def u_stage(c):
            b, j = bj(c)
            u_sb = u_tiles[b]
            js = slice(j * NT, (j + 1) * NT)
            pn = pn_t.pop(c)
            for h in range(2):
                us = ppu.tile([128, NT], f32, tag="u_ps", name=f"us{c}_{h}")
                nc.tensor.matmul(us[:], qa_sb[:, b, h * 128:(h + 1) * 128], pn[:],
                                 start=True, stop=True)
                if h == 0:
                    nc.scalar.copy(u_sb[:, h, js], us[:])
                else:
                    nc.vector.tensor_copy(u_sb[:, h, js], us[:])

        def x_flush(c):
            # flush half a sample per DMA: large transfers, few issues.
            # Last sample flushes per-chunk so the tail drains early
            # (the final flush was 14.7us of pure tail in v3).
            b, j = bj(c)
            if b == BPC - 1 and j >= 4:
                hs = slice(j * NT, (j + 1) * NT)
            elif j % 4 == 3:
                hs = slice((j - 3) * NT, (j + 1) * NT)
            else:
                return
            nc.sync.dma_start(out_v[b, 1, :, :, hs], x_tiles[b][:, :, hs])

        def u_flush(c):
            b, j = bj(c)
            if b == BPC - 1 and j >= 4:
                hs = slice(j * NT, (j + 1) * NT)
            elif j % 4 == 3:
                hs = slice((j - 3) * NT, (j + 1) * NT)
            else:
                return
            nc.gpsimd.dma_start(out_v[b, 0, :, :, hs], u_tiles[b][:, :, hs])

        # Chunk-level software pipeline: stage s of chunk c is emitted at
        # iteration c+s, so every cross-engine hop has a full iteration of
        # slack and the PE stream never waits on the softmax chain.
        # Emission order inside an iteration = engine queue order: oldest
        # work first, so evictions/feeds retire before new dependencies.
        for t in range(T + 7):
            if 0 <= t - 4 < T:
                pn_stage(t - 4)
            if 0 <= t - 5 < T:
                u_stage(t - 5)
            if 0 <= t - 3 < T:
                recip_stage(t - 3)
            if 0 <= t - 2 < T:
                sum_stage(t - 2)
            if 0 <= t - 1 < T:
                logit_stage(t - 1)
            if t < T:
                proj_chunk(t)
            if t == 0:
                load_attn_consts()
            if 0 <= t - 2 < T:
                x_flush(t - 2)
            if 0 <= t - 6 < T:
                u_flush(t - 6)

    nc.compile()
    return nc


def kernel(feats, preds, labels, flag, queue, Wp, bp):
    from concourse.bass_utils import run_bass_kernel_spmd
    global LAST_RESULTS

    feats = np.ascontiguousarray(np.asarray(feats, dtype=np.float32))
    preds = np.ascontiguousarray(np.asarray(preds, dtype=np.float32))
    labels = np.asarray(labels).astype(np.int64)
    queue = np.ascontiguousarray(np.asarray(queue, dtype=np.float32))
    Wp = np.ascontiguousarray(np.asarray(Wp, dtype=np.float32))
    bp = np.ascontiguousarray(np.asarray(bp, dtype=np.float32))
    try:
        flag_v = int(np.asarray(flag))
    except TypeError:
        flag_v = int(flag)

    qfin = _host_queue_update(feats, preds, labels, flag_v, queue, Wp, bp)
    qA = np.ascontiguousarray(qfin[labels].astype(np.float16))      # [B, 20, 256]
    qAT = np.ascontiguousarray(qA.transpose(0, 2, 1))               # [B, 256, 20]
    wpt = np.ascontiguousarray(Wp.T.astype(np.float16))             # [512, 256]
    bpc = np.ascontiguousarray(bp.reshape(2, 128).T)
    ones = np.ones((MEM, MEM), dtype=np.float16)

    if "prog" not in _PROGRAM_CACHE:
        _PROGRAM_CACHE["prog"] = _build_program()
    nc = _PROGRAM_CACHE["prog"]

    f4 = feats.reshape(B, IN_C, HW).astype(np.float16)
    in_maps = []
    for k in range(NCORES):
        s = slice(k * BPC, (k + 1) * BPC)
        in_maps.append({
            "feats": np.ascontiguousarray(f4[s]),
            "wpt": wpt,
            "bpc": bpc,
            "qat": np.ascontiguousarray(qAT[s]),
            "qa": np.ascontiguousarray(qA[s]),
            "ones": ones,
        })

    trace = bool(int(os.environ.get("KERNEL_TRACE", "0")))
    tc_env = os.environ.get("KERNEL_TRACE_CORES", "")
    trace_cores = [int(x) for x in tc_env.split(",") if x] or None
    res = run_bass_kernel_spmd(nc, in_maps, core_ids=list(range(NCORES)),
                               trace=trace, trace_cores=trace_cores)
    LAST_RESULTS = res
    out = np.concatenate([res.results[k]["out"] for k in range(NCORES)], axis=0)
    return out.astype(np.float32).reshape(B, 2 * CODE, H, W_SP)


if __name__ == "__main__":
    d = np.load("/tmp/inputs.npz")
    out = kernel(d["feats"], d["preds"], d["labels"], d["flag"], d["queue"], d["Wp"], d["bp"])
    exp = np.load("/tmp/expected.npy")
    err = np.abs(out - exp)
    print("absmax err:", err.max(), "scale-rel:", err.max() / np.abs(exp).max())
